# revision 8
# baseline (speedup 1.0000x reference)
"""PatchAttacker kernel for 8 Trainium2 NeuronCores.

Pastes a bilinearly-resized 512x512x3 patch into 32 images at up to 6 box
locations each (later boxes overwrite earlier ones), matching reference.py.

Strategy (data-parallel over batch, 4 images per core):
  - Host precomputes, from `boxes` only (tiny data): per-region integer
    windows (y0,x0,h,w), bilinear source indices (yl / xl pairs) and lerp
    weights, plus a rectangle decomposition that makes all DRAM region
    writes disjoint (later regions clip earlier ones).
  - Device per region:
      1. gpsimd.dma_gather of row PAIRS from a host-materialized
         `patch_pairs[r] = concat(patch[r], patch[r+1])` tensor: one
         descriptor per output row, 12KB each -> [128, C, 3072] tile with
         output row j on partition j%128, chunk j//128.
      2. y-lerp: ACT does lo*(1-wy) (per-partition scale), DVE does hi*wy
         and the add -> rows [128, C, 1536]  (full 512-wide resampled rows)
      3. x-gather along the free dim with gpsimd.ap_gather (indices are
         lo-block then hi-block) -> [128, C, 2w, 3]
      4. x-lerp: weights vary along free dim, so they are broadcast to all
         128 partitions via a K=1 PE matmul (ones^T @ wgt_row) into PSUM,
         copied to SBUF by ACT; DVE does mult + block add -> [128, C, w*3]
      5. exact-rectangle DMA writes into the output image, ordered after
         the base image copy via explicit deps.
  - The base image copy (input -> output DRAM) runs as 4 big D2D DMAs.
  - Per-core code differs (shapes/offsets are compile-time baked), so each
    core's region code sits in a `tc.If(partition_id == k)` block; all
    per-core data (indices, weights, images) arrives via per-core in_maps.
"""

import sys
import numpy as np

sys.path.insert(0, "/opt/trn_rl_repo")

B, N, H, W, PN = 32, 6, 512, 512, 512
SCALE, ASPECT, MIN_PH = 0.5, 1.0, 60.0
NCORES = 8
BPC = B // NCORES  # images per core
W3 = W * 3
ROW_ELEMS = PN * 3          # 1536 f32 per patch row
PAIR_ELEMS = 2 * ROW_ELEMS  # 3072

_f = np.float32

# dtype of the gathered patch-pair data; "float16" halves the dominant
# HBM gather traffic at ~5e-4 relative error (f32 path is ~1e-5).
PATCH_DT = "float32"


# ----------------------------------------------------------------------------
# Host-side reference-exact box math
# ----------------------------------------------------------------------------

def _patch_boxes_np(boxes):
    ymin, xmin, ymax, xmax = (boxes[..., 0], boxes[..., 1],
                              boxes[..., 2], boxes[..., 3])
    h = ymax - ymin
    w = xmax - xmin
    pw = h * _f(SCALE)
    ph = _f(ASPECT) * pw
    oy = ymin + h * _f(0.5)
    ox = xmin + w * _f(0.5)
    yp = np.maximum(oy - ph * _f(0.5), _f(0.0))
    xp = np.maximum(ox - pw * _f(0.5), _f(0.0))
    yp = np.where(yp + ph > _f(H), _f(H) - ph, yp)
    xp = np.where(xp + pw > _f(W), _f(W) - pw, xp)
    valid = ph > _f(MIN_PH)
    pb = np.stack([yp, xp, ph, pw], axis=-1).astype(np.int32)  # trunc cast
    return pb, valid


def _axis_samples(n):
    """Bilinear source indices/weights for resizing PN -> n (f32-exact)."""
    d = np.arange(n).astype(np.float32)
    s = _f(PN) / _f(n)
    sv = np.clip((d + _f(0.5)) * s - _f(0.5), _f(0.0), _f(PN - 1.0))
    lo = np.floor(sv).astype(np.int32)
    wgt = (sv - lo.astype(np.float32)).astype(np.float32)
    return lo, wgt


def _sub_rect(p, q):
    """Rectangle p minus rectangle q -> list of disjoint rects.
    Rects are (y0, y1, x0, x1), half-open."""
    py0, py1, px0, px1 = p
    qy0, qy1, qx0, qx1 = q
    iy0, iy1 = max(py0, qy0), min(py1, qy1)
    ix0, ix1 = max(px0, qx0), min(px1, qx1)
    if iy0 >= iy1 or ix0 >= ix1:
        return [p]
    out = []
    if py0 < iy0:
        out.append((py0, iy0, px0, px1))
    if iy1 < py1:
        out.append((iy1, py1, px0, px1))
    if px0 < ix0:
        out.append((iy0, iy1, px0, ix0))
    if ix1 < px1:
        out.append((iy0, iy1, ix1, px1))
    return out


def _cdiv(a, b):
    return -(-a // b)


def _r16(a):
    return _cdiv(a, 16) * 16


class _Region:
    __slots__ = ("img", "y0", "x0", "h", "w", "C", "yoff", "xoff", "ccol",
                 "wrow", "pieces", "n16")


def _make_plans(boxes):
    """Per-core region plans + packed index/weight arrays (uniform shapes)."""
    pb, valid = _patch_boxes_np(boxes)
    plans = []
    for k in range(NCORES):
        regions = []
        for bi in range(BPC):
            b = k * BPC + bi
            img_regions = []
            for n in range(N):
                if not bool(valid[b, n]):
                    continue
                y0, x0, h, w = (int(pb[b, n, 0]), int(pb[b, n, 1]),
                                int(pb[b, n, 2]), int(pb[b, n, 3]))
                img_regions.append((n, y0, x0, h, w))
            # visible pieces: each region minus all later regions of same img
            for i, (n, y0, x0, h, w) in enumerate(img_regions):
                pieces = [(y0, y0 + h, x0, x0 + w)]
                for (_, ly, lx, lh, lw) in img_regions[i + 1:]:
                    nxt = []
                    for p in pieces:
                        nxt += _sub_rect(p, (ly, ly + lh, lx, lx + lw))
                    pieces = nxt
                if not pieces:
                    continue
                r = _Region()
                r.img, r.y0, r.x0, r.h, r.w = bi, y0, x0, h, w
                r.C = _cdiv(h, 128)
                r.n16 = _r16(2 * w)
                r.pieces = pieces
                regions.append(r)
        plans.append(regions)

    # packed-array extents (uniform across cores)
    ylen = max(sum(_cdiv(r.h, 16) + (_cdiv(r.h, 16) & 1) for r in p)
               for p in plans) or 1
    xlen = max(sum(r.n16 // 16 + ((r.n16 // 16) & 1) for r in p)
               for p in plans) or 1
    clen = max(sum(r.C for r in p) for p in plans) or 1
    rmax = max(len(p) for p in plans) or 1

    yidx = np.zeros((NCORES, 128, ylen), np.int16)
    xidx = np.zeros((NCORES, 128, xlen), np.int16)
    wyab = np.zeros((NCORES, 128, 2 * clen), np.float32)
    xwgt = np.zeros((NCORES, rmax, 1248), np.float32)

    for k, regions in enumerate(plans):
        yo = xo = co = 0
        for ri, r in enumerate(regions):
            yl, wy = _axis_samples(r.h)
            xl, wx = _axis_samples(r.w)
            r.yoff, r.xoff, r.ccol, r.wrow = yo, xo, co, ri
            # y indices, wrapped by 16, replicated to all 8 groups of 16
            ny = _cdiv(r.h, 16)
            buf = np.zeros(ny * 16, np.int16)
            buf[:r.h] = yl.astype(np.int16)
            buf[r.h:] = -1  # trailing negatives are ignored by dma_gather
            wrap = buf.reshape(ny, 16).T  # [16, ny]
            yidx[k, :, yo:yo + ny] = np.tile(wrap, (8, 1))
            yo += ny + (ny & 1)  # keep 4-byte alignment of idx slices
            # x indices: lo block then hi block, wrapped by 16
            nx = r.n16
            bufx = np.zeros(nx, np.int16)
            bufx[:r.w] = xl.astype(np.int16)
            bufx[r.w:2 * r.w] = np.minimum(xl + 1, PN - 1).astype(np.int16)
            wrapx = bufx.reshape(nx // 16, 16).T  # [16, nx/16]
            xidx[k, :, xo:xo + nx // 16] = np.tile(wrapx, (8, 1))
            xo += nx // 16 + ((nx // 16) & 1)  # 4-byte-aligned slices
            # y weights per chunk: col 2c = 1-wy, col 2c+1 = wy
            for c in range(r.C):
                rc = min(128, r.h - c * 128)
                wyab[k, :rc, 2 * co] = (_f(1.0) - wy[c * 128:c * 128 + rc])
                wyab[k, :rc, 2 * co + 1] = wy[c * 128:c * 128 + rc]
                co += 1
            # x weights: lo block (1-wx)*3 then hi block wx*3
            w3 = r.w * 3
            xwgt[k, ri, 0:w3] = np.repeat(_f(1.0) - wx, 3)
            xwgt[k, ri, w3:2 * w3] = np.repeat(wx, 3)

    dims = dict(ylen=ylen, xlen=xlen, clen=clen, rmax=rmax)
    return plans, yidx, xidx, wyab, xwgt, dims


# ----------------------------------------------------------------------------
# Device program
# ----------------------------------------------------------------------------

def _emit_core(nc, tc, wpool, ppool, regions, sb, copy_insts):
    from concourse import mybir
    from concourse.tile import add_dep_helper
    dt = mybir.dt
    AF = mybir.ActivationFunctionType
    OP = mybir.AluOpType
    yidx_sb, xidx_sb, wy_sb, ones_sb, xwgt_d, imgs_out = sb

    for ri, r in enumerate(regions):
        C, h, w = r.C, r.h, r.w
        n3 = 2 * w * 3

        # 1. gather row pairs: out row j -> partition j%128, chunk j//128
        pdt = getattr(dt, PATCH_DT)
        pairs = wpool.tile([128, C, PAIR_ELEMS], pdt, tag="pairs")
        nc.gpsimd.dma_gather(
            out_ap=pairs[:, :, :],
            in_ap=nc.patch_pairs_t[:, :],
            idxs_ap=yidx_sb[:, r.yoff:r.yoff + _cdiv(h, 16)],
            num_idxs=h,
            num_idxs_reg=h,
            elem_size=PAIR_ELEMS,
            queue_num=0,
        )

        # 2. y-lerp
        rows = wpool.tile([128, C, ROW_ELEMS], dt.float32, tag="rows")
        for c in range(C):
            col = 2 * (r.ccol + c)
            t0 = wpool.tile([128, ROW_ELEMS], dt.float32, tag="t0")
            nc.scalar.activation(
                out=t0[:, :], in_=pairs[:, c, 0:ROW_ELEMS],
                func=AF.Copy, scale=wy_sb[:, col:col + 1])
            nc.vector.tensor_scalar_mul(
                rows[:, c, :], pairs[:, c, ROW_ELEMS:PAIR_ELEMS],
                wy_sb[:, col + 1:col + 2])
            nc.vector.tensor_tensor(
                out=rows[:, c, :], in0=rows[:, c, :], in1=t0[:, :], op=OP.add)

        # 4a. broadcast x-weights across partitions via K=1 matmul
        wrow = wpool.tile([1, n3], dt.float32, tag="wrow")
        nc.sync.dma_start(out=wrow[:, :], in_=xwgt_d[r.wrow:r.wrow + 1, 0:n3])
        wps = ppool.tile([128, n3], dt.float32, tag="wps")
        for a in range(0, n3, 512):
            b_ = min(a + 512, n3)
            nc.tensor.matmul(out=wps[:, a:b_], lhsT=ones_sb[:, :],
                             rhs=wrow[:, a:b_], start=True, stop=True)
        wsb = wpool.tile([128, n3], dt.float32, tag="wsb")
        nc.scalar.activation(out=wsb[:, :], in_=wps[:, :], func=AF.Copy)

        # 3+4b. x-gather + x-lerp per chunk
        fin = wpool.tile([128, C, w * 3], dt.float32, tag="fin")
        for c in range(C):
            xg = wpool.tile([128, r.n16 * 3], dt.float32, tag="xg")
            nc.gpsimd.ap_gather(
                out_ap=xg[:, :], in_ap=rows[:, c, :],
                idxs_ap=xidx_sb[:, r.xoff:r.xoff + r.n16 // 16],
                channels=128, num_elems=PN, d=3, num_idxs=r.n16)
            m = wpool.tile([128, n3], dt.float32, tag="m")
            nc.vector.tensor_tensor(
                out=m[:, :], in0=xg[:, 0:n3], in1=wsb[:, :], op=OP.mult)
            nc.vector.tensor_tensor(
                out=fin[:, c, :], in0=m[:, 0:w * 3], in1=m[:, w * 3:n3],
                op=OP.add)

        # 5. disjoint rectangle writes, ordered after the base copy
        for (py0, py1, px0, px1) in r.pieces:
            for c in range(C):
                cy0, cy1 = r.y0 + c * 128, r.y0 + min(h, (c + 1) * 128)
                a0, a1 = max(py0, cy0), min(py1, cy1)
                if a0 >= a1:
                    continue
                l0, l1 = a0 - cy0, a1 - cy0
                wr = nc.sync.dma_start(
                    out=imgs_out[r.img, a0:a1, px0 * 3:px1 * 3],
                    in_=fin[l0:l1, c, (px0 - r.x0) * 3:(px1 - r.x0) * 3])
                add_dep_helper(wr.ins, copy_insts[r.img].ins,
                               reason="overlay after base image copy")


def _build_program(plans, dims):
    from concourse import bacc, mybir, tile
    dt = mybir.dt
    nc = bacc.Bacc("TRN2", target_bir_lowering=False, debug=False,
                   num_swdge_queues=4)
    nc.num_swdge_queues_used = 4

    imgs_in = nc.dram_tensor("imgs_in", [BPC, H, W3], dt.float32,
                             kind="ExternalInput")
    ppairs = nc.dram_tensor("patch_pairs", [PN, PAIR_ELEMS],
                            getattr(dt, PATCH_DT), kind="ExternalInput")
    yidx_d = nc.dram_tensor("yidx", [128, dims["ylen"]], dt.int16,
                            kind="ExternalInput")
    xidx_d = nc.dram_tensor("xidx", [128, dims["xlen"]], dt.int16,
                            kind="ExternalInput")
    wy_d = nc.dram_tensor("wy", [128, 2 * dims["clen"]], dt.float32,
                          kind="ExternalInput")
    xwgt_d = nc.dram_tensor("xwgt", [dims["rmax"], 1248], dt.float32,
                            kind="ExternalInput")
    imgs_out = nc.dram_tensor("imgs_out", [BPC, H, W3], dt.float32,
                              kind="ExternalOutput")
    nc.patch_pairs_t = ppairs

    with tile.TileContext(nc) as tc:
        pid = nc.partition_id()
        copy_insts = [nc.sync.dma_start(out=imgs_out[i], in_=imgs_in[i])
                      for i in range(BPC)]
        with tc.tile_pool(name="consts", bufs=1) as cpool, \
             tc.tile_pool(name="work", bufs=2) as wpool, \
             tc.tile_pool(name="wpsum", bufs=2, space="PSUM") as ppool:
            yidx_sb = cpool.tile([128, dims["ylen"]], dt.int16)
            nc.sync.dma_start(out=yidx_sb[:, :], in_=yidx_d[:, :])
            xidx_sb = cpool.tile([128, dims["xlen"]], dt.int16)
            nc.sync.dma_start(out=xidx_sb[:, :], in_=xidx_d[:, :])
            wy_sb = cpool.tile([128, 2 * dims["clen"]], dt.float32)
            nc.sync.dma_start(out=wy_sb[:, :], in_=wy_d[:, :])
            ones_sb = cpool.tile([1, 128], dt.float32)
            nc.vector.memset(ones_sb[:, :], 1.0)

            sb = (yidx_sb, xidx_sb, wy_sb, ones_sb, xwgt_d, imgs_out)
            for k in range(NCORES):
                with tc.If(pid == k):
                    _emit_core(nc, tc, wpool, ppool, plans[k], sb, copy_insts)
    nc.finalize()
    return nc


# ----------------------------------------------------------------------------
# Entry points
# ----------------------------------------------------------------------------

def _prepare(images, boxes, patch):
    images = np.ascontiguousarray(images, dtype=np.float32)
    boxes = np.ascontiguousarray(boxes, dtype=np.float32)
    patch = np.ascontiguousarray(patch, dtype=np.float32)

    plans, yidx, xidx, wyab, xwgt, dims = _make_plans(boxes)

    flat = patch.reshape(PN, ROW_ELEMS)
    nxt = np.concatenate([flat[1:], flat[-1:]], axis=0)
    patch_pairs = np.concatenate([flat, nxt], axis=1)  # [512, 3072]
    patch_pairs = patch_pairs.astype(PATCH_DT)

    in_maps = []
    for k in range(NCORES):
        in_maps.append({
            "imgs_in": images[k * BPC:(k + 1) * BPC].reshape(BPC, H, W3),
            "patch_pairs": patch_pairs,
            "yidx": yidx[k],
            "xidx": xidx[k],
            "wy": wyab[k],
            "xwgt": xwgt[k],
        })
    nc = _build_program(plans, dims)
    return nc, in_maps


def run(images, boxes, patch, **kwargs):
    """Build + run; returns (output, BassKernelResults)."""
    from concourse import bass_utils
    nc, in_maps = _prepare(images, boxes, patch)
    res = bass_utils.run_bass_kernel_spmd(
        nc, in_maps, core_ids=list(range(NCORES)), **kwargs)
    out = np.stack([res.results[k]["imgs_out"] for k in range(NCORES)])
    return out.reshape(B, H, W, 3), res


def kernel(images, boxes, patch):
    out, _ = run(images, boxes, patch)
    return out
